# revision 74
# baseline (speedup 1.0000x reference)
"""Trainium2 Bass kernel for nn_Attention_47313359733175.

Vector-neuron style attention: B=8, C=128, N=1024, H=8 heads.
  q/k/v = VNLinear(W, x)  : (B,384,3,N), reshaped to heads of 144 features
  attn  = softmax(q k^T / sqrt(48)), out = VNLinear(Wo, attn v)

Sharding: pure data-parallel over the batch dim; core i computes batch i.

Per-core plan (all on-chip after one input DMA; everything in bf16 except
PSUM accumulation, rel err ~5e-3 vs the 2e-2 gate):
  - Q/K are projected into a per-head *stacked* layout so the score
    contraction (144 feats = 48ch x 3 components) runs as TWO matmuls per
    key chunk instead of three: a main tile [96 = d0(48)|d1(48), n] built
    by two zero-padded accumulated matmuls, and a per-head-pair tail tile
    [d2 of even head @0:48 | pad | d2 of odd head @64:112, n] built by one
    matmul.  Scores S^T = K_blk^T Q_blk then accumulate main (K=96) +
    tail (K=48 at base 0/64) into PSUM.  The zero-padded projection
    weights are built on-chip (GPSIMD memset + block copies) from a
    compact DMA image; X streams in bf16.  Most K projections are
    drip-fed into the first heads' key loops as side jobs so compute
    starts as soon as the first DMA chunks land.
  - exp on ScalarE straight out of PSUM with the 1/sqrt(48) scale folded
    in, writing bf16.  Weights are ~0.05-scale so scores are O(1): no max
    subtraction needed.
  - V is projected transposed (sequence on partitions) into per-head
    segments with 32-aligned component blocks [d0|pad|d1|pad][d2|pad|
    ones@192|pad]; the ones column makes the attn*V matmul also produce
    softmax row sums (psum row 64).  U^T = V_seg^T E accumulates over key
    chunks in PSUM (M=128 main + M=96 tail), one head at a time with the
    accumulators double-buffered across heads; U matmuls lag the score
    matmuls by LAG key chunks so exp latency stays off the PE.
  - The normalize chain (DVE reciprocal -> GPSIMD partition_broadcast ->
    per-block DVE scale) writes straight into pair-grouped U tiles
    (even head rows 0:48, odd head rows 64:112, per component), so the
    output projection runs as ONE K=128 matmul per (pair, component),
    accumulated over pair-pairs in PSUM and drained to an SBUF
    accumulator twice per query half -- 24 matmuls + 12 drains total
    instead of 64 + 48.  Output-projection stages are deferred into the
    next head's key loop so the normalize latency never stalls the PE.
  - The very last head splits its U accumulation and the whole
    normalize -> project -> drain -> DMA chain into query halves so the
    serial stages pipeline into the final (engine-spread) output DMAs.
"""

import sys

sys.path.insert(0, "/opt/trn_rl_repo")

import numpy as np
from contextlib import ExitStack

import concourse.bass as bass
import concourse.bacc as bacc
import concourse.mybir as mybir
import concourse.tile as tile
from concourse.bass import ts, ds
from concourse.bass_utils import run_bass_kernel_spmd

P = 128          # partitions
N = 1024         # sequence length
C = 128          # input channels
F = 384          # projected channels (3C)
NH = 8           # heads
FH = 48          # channels per head
D3 = 3           # vector components
SEG = 224        # per-head V segment, 32-aligned d blocks (bf16):
                 #   [d0@0:48|pad|d1@64:112|pad] [d2@128:176|pad|ones@192|pad]
VW = SEG * NH    # 1792
NCORES = 8
SCALE = float(FH) ** -0.5
LAG = 3          # u_mms lag behind scores (key chunks)

WMAIN = 144      # per-head stacked main proj weights: [zeros 48|Wblk 48|zeros 48]
WTAIL = 112      # per-pair tail proj weights: [Wblk_even 48|zeros 16|Wblk_odd 48]
WQK = NH * WMAIN + (NH // 2) * WTAIL            # 1600 (on-chip padded form)
WCMP = NH * FH + (NH // 2) * 2 * FH             # 768  (compact DMA form)
PACK1W = F + 3 * N + 2 * WCMP                   # WVT, X0..2, WKc, WQc = 4992
PACK2W = (NH // 2) * P                          # pair-stacked out-proj blocks

F32 = mybir.dt.float32
BF16 = mybir.dt.bfloat16


def _build_program():
    nc = bacc.Bacc(
        "TRN2", target_bir_lowering=False, debug=False, enable_asserts=False
    )

    packed = nc.dram_tensor("packed", (P, PACK1W), BF16, kind="ExternalInput")
    wfin = nc.dram_tensor("wfin", (P, PACK2W), BF16, kind="ExternalInput")
    out = nc.dram_tensor("out", (C, D3, N), F32, kind="ExternalOutput")

    with tile.TileContext(nc) as tc:
        with ExitStack() as ctx:
            const = ctx.enter_context(tc.tile_pool(name="const", bufs=1))
            vpool = ctx.enter_context(tc.tile_pool(name="vpool", bufs=1))
            kpool = ctx.enter_context(tc.tile_pool(name="kpool", bufs=1))
            qmp = ctx.enter_context(tc.tile_pool(name="qmp", bufs=2))
            qtp = ctx.enter_context(tc.tile_pool(name="qtp", bufs=2))
            epool = ctx.enter_context(tc.tile_pool(name="epool", bufs=9))
            uapp = ctx.enter_context(tc.tile_pool(name="uapp", bufs=2))
            rrp = ctx.enter_context(tc.tile_pool(name="rrp", bufs=2))
            # PSUM budget: 3 + 5 = 8 banks exactly.  pps: scores double-buffer
            # + in-flight Q-projection psum; ppu: pA/pB (double-buffered
            # across heads) + the three per-head output-projection tiles.
            pps = ctx.enter_context(tc.tile_pool(name="pps", bufs=3, space="PSUM"))
            ppu = ctx.enter_context(tc.tile_pool(name="ppu", bufs=5, space="PSUM"))

            PK = const.tile([P, PACK1W], BF16, name="PK")
            # SBUF layout: [WVT 384 | X0 1024 | X1 1024 | WKc 768 | X2 1024
            #               | WQc 768]; DMA chunk order is chosen so the V/K
            # projections can start as early as possible.
            cWVT = (0, F)
            cX0 = (F, F + N)
            cX1 = (F + N, F + 2 * N)
            cWKc = (F + 2 * N, F + 2 * N + WCMP)
            cX2 = (cWKc[1], cWKc[1] + N)
            cWQc = (cX2[1], PACK1W)
            chunks = [
                (0, F + 128),                      # WVT + X0 m0
                (F + 128, F + 512),                # X0 m1-3
                (cX1[0], cX1[0] + 512),            # X1a
                (F + 512, F + N),                  # X0b
                (cX1[0] + 512, cX1[1]),            # X1b
                cWKc,
                cX2,
                cWQc,
            ]
            dmae = [nc.sync, nc.scalar]
            for i, (lo, hi) in enumerate(chunks):
                dmae[i % 2].dma_start(PK[:, lo:hi], packed.ap()[:, lo:hi])
            WVT = PK[:, cWVT[0] : cWVT[1]]
            Xd = [
                PK[:, cX0[0] : cX0[1]],
                PK[:, cX1[0] : cX1[1]],
                PK[:, cX2[0] : cX2[1]],
            ]

            # zero-padded stacked projection weights, built on-chip from the
            # compact DMA form (saves ~1.7MB of input DMA)
            WKP = const.tile([P, WQK], BF16, name="WKP")
            WQP = const.tile([P, WQK], BF16, name="WQP")
            nc.gpsimd.memset(WKP.bitcast(mybir.dt.uint16)[:], 0)
            nc.gpsimd.memset(WQP.bitcast(mybir.dt.uint16)[:], 0)

            def wmain(wp, h):
                return wp[:, WMAIN * h : WMAIN * (h + 1)]

            def wtail(wp, p):
                o = NH * WMAIN + WTAIL * p
                return wp[:, o : o + WTAIL]

            def build_pads(wp, ccmp, start_eng):
                # SBUF->SBUF block copies on the otherwise-idle GPSIMD engine
                def cpy(dst, lo, w):
                    nc.gpsimd.tensor_copy(
                        out=dst, in_=PK[:, ccmp[0] + lo : ccmp[0] + lo + w])

                for h in range(NH):
                    cpy(wmain(wp, h)[:, 48:96], FH * h, FH)
                for p in range(NH // 2):
                    t = wtail(wp, p)
                    cpy(t[:, 0:48], NH * FH + 96 * p, FH)
                    cpy(t[:, 64:112], NH * FH + 96 * p + FH, FH)

            # pair-stacked output-projection weights: per pair p, rows 0:48 =
            # WoT block of head 2p, rows 64:112 = head 2p+1 (zeros elsewhere);
            # the same tile serves all three output components.
            WFT = const.tile([P, NH // 2, P], BF16, name="WFT")
            nc.sync.dma_start(
                WFT[:], wfin.ap().rearrange("p (j q) -> p j q", j=NH // 2)
            )
            OutSB = const.tile([P, D3, N], F32, name="OutSB")

            # ---- V projection: V_seq[m][:, seg(h)] = (x[:, d, m-slice]^T Wv^T)
            Vseq = [
                vpool.tile([P, VW], BF16, name=f"vs{m}", tag=f"vs{m}")
                for m in range(8)
            ]
            # prologue copies rotate DVE / ScalarE (GPSIMD cannot read PSUM)
            cp = [
                nc.vector.tensor_copy,
                lambda out, in_: nc.scalar.copy(out=out, in_=in_),
            ]
            ci = [0]

            def copy_alt(out, in_):
                cp[ci[0] % 2](out=out, in_=in_)
                ci[0] += 1

            # K stacked tiles: per head main [96, N], per pair tail [112, N]
            Kms = [
                kpool.tile([96, D3 * 0 + N], BF16, name=f"km{h}", tag=f"km{h}")
                for h in range(NH)
            ]
            Kts = [
                kpool.tile([WTAIL, N], BF16, name=f"kt{p}", tag=f"kt{p}")
                for p in range(NH // 2)
            ]
            vrs = []
            VD_OFF = [0, 64, 128]  # 32-aligned per-component block offsets
            for m in range(8):
                vrs.append(Vseq[m].rearrange("p (h s) -> p h s", s=SEG))

            def vseq_pad_init(m):
                vu = Vseq[m].bitcast(mybir.dt.uint16).rearrange(
                    "p (h s) -> p h s", s=SEG
                )
                # zero the pad columns (they feed matmul lhsT), 1.0 ones col
                nc.gpsimd.memset(vu[:, :, 48:64], 0)
                nc.gpsimd.memset(vu[:, :, 112:128], 0)
                nc.gpsimd.memset(vu[:, :, 176:192], 0)
                nc.gpsimd.memset(vu[:, :, 193:224], 0)
                nc.gpsimd.memset(vu[:, :, 192], 0x3F80)

            def v_proj(m, d, pool, tag, eng_copy):
                vr = vrs[m]
                pv = pool.tile([P, F], F32, name=f"pv{m}{d}", tag=tag)
                nc.tensor.matmul(
                    pv[:], lhsT=Xd[d][:, ts(m, P)], rhs=WVT[:],
                    start=True, stop=True,
                )
                pvh = pv.rearrange("p (h f) -> p h f", f=FH)
                eng_copy(out=vr[:, :, VD_OFF[d] : VD_OFF[d] + 48], in_=pvh)

            # stacked main projection: two zero-padded accumulated matmuls
            # (d0 then d1) into one [96, 512] psum tile.
            def proj_main(wbase, h, nsl, pool, tag):
                pm = pool.tile([96, 512], F32, name=f"pm{tag}{h}", tag=tag)
                wm = wmain(wbase, h)
                nc.tensor.matmul(
                    pm[:], lhsT=wm[:, 48:144], rhs=Xd[0][:, nsl],
                    start=True, stop=False,
                )
                nc.tensor.matmul(
                    pm[:], lhsT=wm[:, 0:96], rhs=Xd[1][:, nsl],
                    start=False, stop=True,
                )
                return pm

            def proj_tail(wbase, p, nsl, pool, tag):
                pt = pool.tile([WTAIL, 512], F32, name=f"pt{tag}{p}", tag=tag)
                nc.tensor.matmul(
                    pt[:], lhsT=wtail(wbase, p)[:], rhs=Xd[2][:, nsl],
                    start=True, stop=True,
                )
                return pt

            def k_proj_main(h, half, pool, tag, eng_copy):
                pm = proj_main(WKP, h, ts(half, 512), pool, tag)
                eng_copy(out=Kms[h][:, ts(half, 512)], in_=pm[:])

            def k_proj_tail(p, half, pool, tag, eng_copy):
                pt = proj_tail(WKP, p, ts(half, 512), pool, tag)
                eng_copy(out=Kts[p][:, ts(half, 512)], in_=pt[:])

            # prologue: V d0/d1 first (they only need X0/X1), then the WK pad
            # build + K projections for heads 0/1 and tail pair 0; the
            # remaining K projections are injected into the first heads' key
            # loops as side jobs.
            ppools = [ppu, pps]
            for d in range(2):
                for m in range(8):
                    v_proj(m, d, ppools[m % 2], "pu" if m % 2 == 0 else "ps",
                           copy_alt)
            build_pads(WKP, cWKc, 0)
            ki = [0]

            def k_job(h_or_p, half, tail=False):
                pool = ppools[ki[0] % 2]
                tag = "pu" if ki[0] % 2 == 0 else "ps"
                ki[0] += 1
                if tail:
                    k_proj_tail(h_or_p, half, pool, tag, copy_alt)
                else:
                    k_proj_main(h_or_p, half, pool, tag, copy_alt)

            for h in (0, 1):
                for half in range(2):
                    k_job(h, half)
            for m in range(8):
                v_proj(m, 2, ppools[m % 2], "pu" if m % 2 == 0 else "ps",
                       copy_alt)
            for half in range(2):
                k_job(0, half, tail=True)
            build_pads(WQP, cWQc, 1)
            for m in range(8):
                vseq_pad_init(m)
            # deadline-ordered side jobs: main halves for heads 2-7 and tail
            # pairs 1-3, drip-fed 3 per head into heads 0-5 of nch 0.
            side_jobs = []
            for grp in ((2, 3, 1), (4, 5, 2), (6, 7, 3)):
                for h in grp[:2]:
                    for half in range(2):
                        side_jobs.append((h, half, False))
                for half in range(2):
                    side_jobs.append((grp[2], half, True))
            sji = [0]

            def run_side_job():
                if sji[0] < len(side_jobs):
                    h_or_p, half, tl = side_jobs[sji[0]]
                    sji[0] += 1
                    k_job(h_or_p, half, tail=tl)

            # ---- main loop: 2 query halves x 8 heads.  Q projection for the
            # next head is issued *inside* the current head's key loop (psum
            # at m==3, tail at m==5) so the Q tiles are ready at the head
            # boundary; the previous head's output projection is deferred to
            # m==4 so its normalize chain never stalls the PE.
            def q_proj_main(nch_, h):
                nsl = ds(512 * nch_, 512)
                Qm = qmp.tile([96, 512], BF16, name=f"qm{nch_}{h}", tag="qm")
                pm = proj_main(WQP, h, nsl, pps, "ps")
                nc.scalar.copy(out=Qm[:], in_=pm[:])
                return Qm

            def q_proj_tail(nch_, h):
                nsl = ds(512 * nch_, 512)
                Qt = qtp.tile([WTAIL, 512], BF16, name=f"qt{nch_}{h}", tag="qt")
                pt = proj_tail(WQP, h // 2, nsl, pps, "ps")
                nc.scalar.copy(out=Qt[:], in_=pt[:])
                return Qt

            heads = [(nch, h) for nch in range(2) for h in range(NH)]
            Qcur = q_proj_main(0, 0)
            Qts = [q_proj_tail(0, 0), None]  # tail tiles by pair parity
            Qnext = [None]
            pending_out = [None]
            UAPs = [None, None]  # pair-grouped U tiles by pair parity

            for hi, (nch, h) in enumerate(heads):
                Qm = Qcur
                Qt = Qts[(h // 2) % 2]
                tb = 64 * (h % 2)
                Km, Kt = Kms[h], Kts[h // 2]
                nxt = heads[hi + 1] if hi + 1 < len(heads) else None

                pA = ppu.tile([P, 512], F32, name=f"pa{nch}{h}", tag="pu")
                pB = ppu.tile([P, 512], F32, name=f"pb{nch}{h}", tag="pu")

                last_head = hi == len(heads) - 1

                def u_mms(m, Em, g=None, pA=pA, pB=pB, h=h):
                    gg = ds(0, 512) if g is None else g
                    nc.tensor.matmul(
                        pA[:, gg],
                        lhsT=Vseq[m][:, SEG * h : SEG * h + 128],
                        rhs=Em[:, gg],
                        start=(m == 0), stop=(m == 7),
                        skip_group_check=g is not None,
                    )
                    # [d2 48 | pad | ones@64 | pad]: row 64 = softmax row sums
                    nc.tensor.matmul(
                        pB[0:96, gg],
                        lhsT=Vseq[m][:, SEG * h + 128 : SEG * h + 224],
                        rhs=Em[:, gg],
                        start=(m == 0), stop=(m == 7),
                        skip_group_check=g is not None,
                    )

                Eq = []
                for m in range(8):
                    pS = pps.tile([P, 512], F32, name=f"s{nch}{h}{m}", tag="ps")
                    nc.tensor.matmul(
                        pS[:], lhsT=Km[:, ts(m, P)], rhs=Qm[:],
                        start=True, stop=False,
                    )
                    nc.tensor.matmul(
                        pS[:], lhsT=Kt[tb : tb + 48, ts(m, P)],
                        rhs=Qt[tb : tb + 48, :],
                        start=False, stop=True,
                    )
                    E = epool.tile([P, 512], BF16, name=f"e{nch}{h}{m}", tag="e")
                    nc.scalar.activation(
                        E[:], pS[:], mybir.ActivationFunctionType.Exp,
                        scale=SCALE,
                    )
                    Eq.append(E)
                    if m in (0, 2, 6):
                        run_side_job()
                    if m == 3 and nxt is not None:
                        Qnext[0] = q_proj_main(*nxt)
                    if m == 4 and pending_out[0] is not None:
                        pending_out[0]()
                        pending_out[0] = None
                    if m == 5 and nxt is not None and nxt[1] % 2 == 0:
                        Qts[(nxt[1] // 2) % 2] = q_proj_tail(*nxt)
                    if m >= LAG:
                        u_mms(m - LAG, Eq[m - LAG],
                              ds(0, 256) if last_head else None)

                # normalize: reciprocal of row sums (pB row 64), broadcast
                # across partitions, then scale each 32-aligned d block into
                # the pair-grouped UAP tiles: rows 0:48 = even head, 64:112 =
                # odd head.  All sources/destinations sit at 32-aligned
                # partition bases.
                pr, sb = h // 2, 64 * (h % 2)
                if h % 2 == 0:
                    UAPc = [
                        uapp.tile([P, 512], BF16, name=f"up{nch}{pr}{d}",
                                  tag=f"up{d}{pr % 2}")
                        for d in range(3)
                    ]
                    UAPs[pr % 2] = UAPc
                UAP = UAPs[pr % 2]
                halves = [ds(0, 512)] if not last_head else [
                    ds(0, 256), ds(256, 256)]
                rr = rrp.tile([P, 512], F32, name=f"rr{nch}{h}", tag="rr")
                Rsb = rrp.tile([P, 512], F32, name=f"rs{nch}{h}", tag="rs")

                def normalize(g, pA=pA, pB=pB, UAP=UAP, sb=sb):
                    nc.vector.reciprocal(out=rr[0:1, g], in_=pB[64:65, g])
                    nc.gpsimd.partition_broadcast(Rsb[:, g], rr[0:1, g])
                    # 64-row blocks: rows 48:64 of each source are zero (the
                    # V-segment pad columns), giving the UAP pad rows true
                    # zeros at no extra cost.
                    for d, (src, lo) in enumerate(
                        [(pA, 0), (pA, 64), (pB, 0)]
                    ):
                        nc.vector.tensor_mul(
                            out=UAP[d][sb : sb + 64, g],
                            in0=src[lo : lo + 64, g], in1=Rsb[0:64, g],
                        )

                # pair-grouped output projection: after pairs (0,1) and
                # (2,3) complete, accumulate Wo over both pairs in PSUM and
                # drain into the SBUF output accumulator.
                def out_stage(nch, stage, UAP2, gs=(ds(0, 512),)):
                    for g in gs:
                        pO = [
                            ppu.tile([P, 512], F32, name=f"po{nch}{stage}{d}",
                                     tag="pu")
                            for d in range(3)
                        ]
                        for d in range(3):
                            for j in range(2):
                                nc.tensor.matmul(
                                    pO[d][:, g], lhsT=WFT[:, 2 * stage + j, :],
                                    rhs=UAP2[j][d][:, g],
                                    start=(j == 0), stop=(j == 1),
                                )
                        eng_dma = [nc.sync, nc.scalar, nc.gpsimd]
                        for d in range(3):
                            osl = OutSB[:, d, ds(512 * nch, 512)][:, g]
                            if stage == 0:
                                nc.vector.tensor_copy(out=osl, in_=pO[d][:, g])
                            else:
                                nc.vector.tensor_add(
                                    out=osl, in0=osl, in1=pO[d][:, g])
                                eng_dma[d].dma_start(
                                    out.ap()[:, d, ds(512 * nch, 512)][:, g],
                                    osl,
                                )

                if not last_head:
                    for m in range(8 - LAG, 8):
                        u_mms(m, Eq[m])
                    Qcur = Qnext[0]
                    normalize(halves[0])
                else:
                    # finish the first query half early, overlap its
                    # normalize with the second half's U accumulation
                    for m in range(8 - LAG, 8):
                        u_mms(m, Eq[m], halves[0])
                    normalize(halves[0])
                    for m in range(8):
                        u_mms(m, Eq[m], halves[1])
                    normalize(halves[1])
                if h % 4 == 3:  # pairs (0,1) done at h=3, (2,3) at h=7
                    UAP2 = [UAPs[(pr - 1) % 2], UAPs[pr % 2]]
                    if not last_head:
                        def out_proj(nch=nch, stage=pr // 2, UAP2=UAP2):
                            out_stage(nch, stage, UAP2)
                        pending_out[0] = out_proj
                    else:
                        out_stage(nch, pr // 2, UAP2, gs=halves)

    nc.compile()
    return nc


def _prep_weights(Wq, Wk, Wv, Wo):
    def compact_qk(W):
        # [8 head main blocks (48 each) | 4 pair tail blocks (48|48)] — both
        # sections happen to be W^T in natural column order; the kernel
        # builds the zero-padded stacked form on-chip.
        Wt = np.ascontiguousarray(W.T).astype(np.float32)  # (128 c, 384 o)
        return np.concatenate([Wt, Wt], axis=1)  # (P, WCMP)

    WoT = np.ascontiguousarray(Wo.T).astype(np.float32)  # (384 o, 128 co)
    # pair-stacked: rows 0:48 = even head's block, 64:112 = odd head's
    wf = np.zeros((P, NH // 2, P), np.float32)
    for p in range(NH // 2):
        wf[0:48, p] = WoT[FH * 2 * p : FH * 2 * p + FH]
        wf[64:112, p] = WoT[FH * (2 * p + 1) : FH * (2 * p + 1) + FH]
    return (
        compact_qk(Wq),
        compact_qk(Wk),
        np.ascontiguousarray(Wv.T).astype(np.float32),
        np.ascontiguousarray(wf.reshape(P, (NH // 2) * P)),
    )


_CACHED_NC = None


def _make_in_maps(vn_x, Wq, Wk, Wv, Wo):
    import ml_dtypes

    wqs, wks, wvt, wf = _prep_weights(
        np.asarray(Wq), np.asarray(Wk), np.asarray(Wv), np.asarray(Wo)
    )
    wf = np.ascontiguousarray(wf.astype(ml_dtypes.bfloat16))
    vn_x = np.asarray(vn_x)
    maps = []
    for b in range(NCORES):
        xb = vn_x[b]
        # SBUF/DRAM layout: [WVT | X0 | X1 | WKc | X2 | WQc]
        packed = np.concatenate(
            [wvt, xb[:, 0], xb[:, 1], wks, xb[:, 2], wqs], axis=1
        ).astype(ml_dtypes.bfloat16)
        assert packed.shape == (P, PACK1W), packed.shape
        maps.append(
            {"packed": np.ascontiguousarray(packed), "wfin": wf}
        )
    return maps


def kernel(vn_x, Wq, Wk, Wv, Wo):
    global _CACHED_NC
    if _CACHED_NC is None:
        _CACHED_NC = _build_program()
    nc = _CACHED_NC

    in_maps = _make_in_maps(vn_x, Wq, Wk, Wv, Wo)
    res = run_bass_kernel_spmd(nc, in_maps, core_ids=list(range(NCORES)))
    out = np.stack([res.results[b]["out"] for b in range(NCORES)])
    return out


# revision 80
# speedup vs baseline: 1.0056x; 1.0056x over previous
"""Trainium2 Bass kernel for nn_Attention_47313359733175.

Vector-neuron style attention: B=8, C=128, N=1024, H=8 heads.
  q/k/v = VNLinear(W, x)  : (B,384,3,N), reshaped to heads of 144 features
  attn  = softmax(q k^T / sqrt(48)), out = VNLinear(Wo, attn v)

Sharding: pure data-parallel over the batch dim; core i computes batch i.

Per-core plan (all on-chip after one input DMA; everything in bf16 except
PSUM accumulation, rel err ~5e-3 vs the 2e-2 gate):
  - Q/K are projected into a per-head *stacked* layout so the score
    contraction (144 feats = 48ch x 3 components) runs as TWO matmuls per
    key chunk instead of three: a main tile [96 = d0(48)|d1(48), n] built
    by two zero-padded accumulated matmuls, and a per-head-pair tail tile
    [d2 of even head @0:48 | pad | d2 of odd head @64:112, n] built by one
    matmul.  Scores S^T = K_blk^T Q_blk then accumulate main (K=96) +
    tail (K=48 at base 0/64) into PSUM.  The zero-padded projection
    weights are built on-chip (GPSIMD memset + block copies) from a
    compact DMA image; X streams in bf16.  Most K projections are
    drip-fed into the first heads' key loops as side jobs so compute
    starts as soon as the first DMA chunks land.
  - exp on ScalarE straight out of PSUM with the 1/sqrt(48) scale folded
    in, writing bf16.  Weights are ~0.05-scale so scores are O(1): no max
    subtraction needed.
  - V is projected transposed (sequence on partitions) into per-head
    segments with 32-aligned component blocks [d0|pad|d1|pad][d2|pad|
    ones@192|pad]; the ones column makes the attn*V matmul also produce
    softmax row sums (psum row 64).  U^T = V_seg^T E accumulates over key
    chunks in PSUM (M=128 main + M=96 tail), one head at a time with the
    accumulators double-buffered across heads; U matmuls lag the score
    matmuls by LAG key chunks so exp latency stays off the PE.
  - The normalize chain (DVE reciprocal -> GPSIMD partition_broadcast ->
    per-block DVE scale) writes straight into pair-grouped U tiles
    (even head rows 0:48, odd head rows 64:112, per component), so the
    output projection runs as ONE K=128 matmul per (pair, component),
    accumulated over pair-pairs in PSUM and drained to an SBUF
    accumulator twice per query half -- 24 matmuls + 12 drains total
    instead of 64 + 48.  Output-projection stages are deferred into the
    next head's key loop so the normalize latency never stalls the PE.
  - The final output-projection stage drains straight into per-component
    DMAs spread across the SP/Act/GPSIMD sequencers to shorten the tail.
"""

import sys

sys.path.insert(0, "/opt/trn_rl_repo")

import numpy as np
from contextlib import ExitStack

import concourse.bass as bass
import concourse.bacc as bacc
import concourse.mybir as mybir
import concourse.tile as tile
from concourse.bass import ts, ds
from concourse.bass_utils import run_bass_kernel_spmd

P = 128          # partitions
N = 1024         # sequence length
C = 128          # input channels
F = 384          # projected channels (3C)
NH = 8           # heads
FH = 48          # channels per head
D3 = 3           # vector components
SEG = 224        # per-head V segment, 32-aligned d blocks (bf16):
                 #   [d0@0:48|pad|d1@64:112|pad] [d2@128:176|pad|ones@192|pad]
VW = SEG * NH    # 1792
NCORES = 8
SCALE = float(FH) ** -0.5
LAG = 3          # u_mms lag behind scores (key chunks)

WMAIN = 144      # per-head stacked main proj weights: [zeros 48|Wblk 48|zeros 48]
WTAIL = 112      # per-pair tail proj weights: [Wblk_even 48|zeros 16|Wblk_odd 48]
WQK = NH * WMAIN + (NH // 2) * WTAIL            # 1600 (on-chip padded form)
WCMP = NH * FH + (NH // 2) * 2 * FH             # 768  (compact DMA form)
PACK1W = F + 3 * N + 2 * WCMP                   # WVT, X0..2, WKc, WQc = 4992
PACK2W = (NH // 2) * P                          # pair-stacked out-proj blocks

F32 = mybir.dt.float32
BF16 = mybir.dt.bfloat16


def _build_program():
    nc = bacc.Bacc(
        "TRN2", target_bir_lowering=False, debug=False, enable_asserts=False
    )

    packed = nc.dram_tensor("packed", (P, PACK1W), BF16, kind="ExternalInput")
    wfin = nc.dram_tensor("wfin", (P, PACK2W), BF16, kind="ExternalInput")
    out = nc.dram_tensor("out", (C, D3, N), F32, kind="ExternalOutput")

    with tile.TileContext(nc) as tc:
        with ExitStack() as ctx:
            const = ctx.enter_context(tc.tile_pool(name="const", bufs=1))
            vpool = ctx.enter_context(tc.tile_pool(name="vpool", bufs=1))
            kpool = ctx.enter_context(tc.tile_pool(name="kpool", bufs=1))
            qmp = ctx.enter_context(tc.tile_pool(name="qmp", bufs=2))
            qtp = ctx.enter_context(tc.tile_pool(name="qtp", bufs=2))
            epool = ctx.enter_context(tc.tile_pool(name="epool", bufs=9))
            uapp = ctx.enter_context(tc.tile_pool(name="uapp", bufs=2))
            rrp = ctx.enter_context(tc.tile_pool(name="rrp", bufs=2))
            # PSUM budget: 3 + 5 = 8 banks exactly.  pps: scores double-buffer
            # + in-flight Q-projection psum; ppu: pA/pB (double-buffered
            # across heads) + the three per-head output-projection tiles.
            pps = ctx.enter_context(tc.tile_pool(name="pps", bufs=3, space="PSUM"))
            ppu = ctx.enter_context(tc.tile_pool(name="ppu", bufs=5, space="PSUM"))

            PK = const.tile([P, PACK1W], BF16, name="PK")
            # SBUF layout: [WVT 384 | X0 1024 | X1 1024 | WKc 768 | X2 1024
            #               | WQc 768]; DMA chunk order is chosen so the V/K
            # projections can start as early as possible.
            cWVT = (0, F)
            cX0 = (F, F + N)
            cX1 = (F + N, F + 2 * N)
            cWKc = (F + 2 * N, F + 2 * N + WCMP)
            cX2 = (cWKc[1], cWKc[1] + N)
            cWQc = (cX2[1], PACK1W)
            chunks = [
                (0, F + 128),                      # WVT + X0 m0
                (F + 128, F + 512),                # X0 m1-3
                (cX1[0], cX1[0] + 512),            # X1a
                (F + 512, F + N),                  # X0b
                (cX1[0] + 512, cX1[1]),            # X1b
                cWKc,
                cX2,
                cWQc,
            ]
            dmae = [nc.sync, nc.scalar]
            for i, (lo, hi) in enumerate(chunks):
                dmae[i % 2].dma_start(PK[:, lo:hi], packed.ap()[:, lo:hi])
            WVT = PK[:, cWVT[0] : cWVT[1]]
            Xd = [
                PK[:, cX0[0] : cX0[1]],
                PK[:, cX1[0] : cX1[1]],
                PK[:, cX2[0] : cX2[1]],
            ]

            # zero-padded stacked projection weights, built on-chip from the
            # compact DMA form (saves ~1.7MB of input DMA)
            WKP = const.tile([P, WQK], BF16, name="WKP")
            WQP = const.tile([P, WQK], BF16, name="WQP")
            nc.gpsimd.memset(WKP.bitcast(mybir.dt.uint16)[:], 0)
            nc.gpsimd.memset(WQP.bitcast(mybir.dt.uint16)[:], 0)

            def wmain(wp, h):
                return wp[:, WMAIN * h : WMAIN * (h + 1)]

            def wtail(wp, p):
                o = NH * WMAIN + WTAIL * p
                return wp[:, o : o + WTAIL]

            def build_pads(wp, ccmp, start_eng):
                # SBUF->SBUF block copies on the otherwise-idle GPSIMD engine
                def cpy(dst, lo, w):
                    nc.gpsimd.tensor_copy(
                        out=dst, in_=PK[:, ccmp[0] + lo : ccmp[0] + lo + w])

                for h in range(NH):
                    cpy(wmain(wp, h)[:, 48:96], FH * h, FH)
                for p in range(NH // 2):
                    t = wtail(wp, p)
                    cpy(t[:, 0:48], NH * FH + 96 * p, FH)
                    cpy(t[:, 64:112], NH * FH + 96 * p + FH, FH)

            # pair-stacked output-projection weights: per pair p, rows 0:48 =
            # WoT block of head 2p, rows 64:112 = head 2p+1 (zeros elsewhere);
            # the same tile serves all three output components.
            WFT = const.tile([P, NH // 2, P], BF16, name="WFT")
            nc.sync.dma_start(
                WFT[:], wfin.ap().rearrange("p (j q) -> p j q", j=NH // 2)
            )
            OutSB = const.tile([P, D3, N], F32, name="OutSB")

            # ---- V projection: V_seq[m][:, seg(h)] = (x[:, d, m-slice]^T Wv^T)
            Vseq = [
                vpool.tile([P, VW], BF16, name=f"vs{m}", tag=f"vs{m}")
                for m in range(8)
            ]
            # prologue copies rotate DVE / ScalarE (GPSIMD cannot read PSUM)
            cp = [
                nc.vector.tensor_copy,
                lambda out, in_: nc.scalar.copy(out=out, in_=in_),
            ]
            ci = [0]

            def copy_alt(out, in_):
                cp[ci[0] % 2](out=out, in_=in_)
                ci[0] += 1

            # K stacked tiles: per head main [96, N], per pair tail [112, N]
            Kms = [
                kpool.tile([96, D3 * 0 + N], BF16, name=f"km{h}", tag=f"km{h}")
                for h in range(NH)
            ]
            Kts = [
                kpool.tile([WTAIL, N], BF16, name=f"kt{p}", tag=f"kt{p}")
                for p in range(NH // 2)
            ]
            vrs = []
            VD_OFF = [0, 64, 128]  # 32-aligned per-component block offsets
            for m in range(8):
                vrs.append(Vseq[m].rearrange("p (h s) -> p h s", s=SEG))

            def vseq_pad_init(m):
                vu = Vseq[m].bitcast(mybir.dt.uint16).rearrange(
                    "p (h s) -> p h s", s=SEG
                )
                # zero the pad columns (they feed matmul lhsT), 1.0 ones col
                nc.gpsimd.memset(vu[:, :, 48:64], 0)
                nc.gpsimd.memset(vu[:, :, 112:128], 0)
                nc.gpsimd.memset(vu[:, :, 176:192], 0)
                nc.gpsimd.memset(vu[:, :, 193:224], 0)
                nc.gpsimd.memset(vu[:, :, 192], 0x3F80)

            def v_proj(m, d, pool, tag, eng_copy):
                vr = vrs[m]
                pv = pool.tile([P, F], F32, name=f"pv{m}{d}", tag=tag)
                nc.tensor.matmul(
                    pv[:], lhsT=Xd[d][:, ts(m, P)], rhs=WVT[:],
                    start=True, stop=True,
                )
                pvh = pv.rearrange("p (h f) -> p h f", f=FH)
                eng_copy(out=vr[:, :, VD_OFF[d] : VD_OFF[d] + 48], in_=pvh)

            # stacked main projection: two zero-padded accumulated matmuls
            # (d0 then d1) into one [96, 512] psum tile.
            def proj_main(wbase, h, nsl, pool, tag):
                pm = pool.tile([96, 512], F32, name=f"pm{tag}{h}", tag=tag)
                wm = wmain(wbase, h)
                nc.tensor.matmul(
                    pm[:], lhsT=wm[:, 48:144], rhs=Xd[0][:, nsl],
                    start=True, stop=False,
                )
                nc.tensor.matmul(
                    pm[:], lhsT=wm[:, 0:96], rhs=Xd[1][:, nsl],
                    start=False, stop=True,
                )
                return pm

            def proj_tail(wbase, p, nsl, pool, tag):
                pt = pool.tile([WTAIL, 512], F32, name=f"pt{tag}{p}", tag=tag)
                nc.tensor.matmul(
                    pt[:], lhsT=wtail(wbase, p)[:], rhs=Xd[2][:, nsl],
                    start=True, stop=True,
                )
                return pt

            def k_proj_main(h, half, pool, tag, eng_copy):
                pm = proj_main(WKP, h, ts(half, 512), pool, tag)
                eng_copy(out=Kms[h][:, ts(half, 512)], in_=pm[:])

            def k_proj_tail(p, half, pool, tag, eng_copy):
                pt = proj_tail(WKP, p, ts(half, 512), pool, tag)
                eng_copy(out=Kts[p][:, ts(half, 512)], in_=pt[:])

            # prologue: V d0/d1 first (they only need X0/X1), then the WK pad
            # build + K projections for heads 0/1 and tail pair 0; the
            # remaining K projections are injected into the first heads' key
            # loops as side jobs.
            ppools = [ppu, pps]
            for d in range(2):
                for m in range(8):
                    v_proj(m, d, ppools[m % 2], "pu" if m % 2 == 0 else "ps",
                           copy_alt)
            build_pads(WKP, cWKc, 0)
            ki = [0]

            def k_job(h_or_p, half, tail=False):
                pool = ppools[ki[0] % 2]
                tag = "pu" if ki[0] % 2 == 0 else "ps"
                ki[0] += 1
                if tail:
                    k_proj_tail(h_or_p, half, pool, tag, copy_alt)
                else:
                    k_proj_main(h_or_p, half, pool, tag, copy_alt)

            for h in (0, 1):
                for half in range(2):
                    k_job(h, half)
            for m in range(8):
                v_proj(m, 2, ppools[m % 2], "pu" if m % 2 == 0 else "ps",
                       copy_alt)
            for half in range(2):
                k_job(0, half, tail=True)
            build_pads(WQP, cWQc, 1)
            for m in range(8):
                vseq_pad_init(m)
            # deadline-ordered side jobs: main halves for heads 2-7 and tail
            # pairs 1-3, drip-fed 3 per head into heads 0-5 of nch 0.
            side_jobs = []
            for grp in ((2, 3, 1), (4, 5, 2), (6, 7, 3)):
                for h in grp[:2]:
                    for half in range(2):
                        side_jobs.append((h, half, False))
                for half in range(2):
                    side_jobs.append((grp[2], half, True))
            sji = [0]

            def run_side_job():
                if sji[0] < len(side_jobs):
                    h_or_p, half, tl = side_jobs[sji[0]]
                    sji[0] += 1
                    k_job(h_or_p, half, tail=tl)

            # ---- main loop: 2 query halves x 8 heads.  Q projection for the
            # next head is issued *inside* the current head's key loop (psum
            # at m==3, tail at m==5) so the Q tiles are ready at the head
            # boundary; the previous head's output projection is deferred to
            # m==4 so its normalize chain never stalls the PE.
            def q_proj_main(nch_, h):
                nsl = ds(512 * nch_, 512)
                Qm = qmp.tile([96, 512], BF16, name=f"qm{nch_}{h}", tag="qm")
                pm = proj_main(WQP, h, nsl, pps, "ps")
                nc.scalar.copy(out=Qm[:], in_=pm[:])
                return Qm

            def q_proj_tail(nch_, h):
                nsl = ds(512 * nch_, 512)
                Qt = qtp.tile([WTAIL, 512], BF16, name=f"qt{nch_}{h}", tag="qt")
                pt = proj_tail(WQP, h // 2, nsl, pps, "ps")
                nc.scalar.copy(out=Qt[:], in_=pt[:])
                return Qt

            heads = [(nch, h) for nch in range(2) for h in range(NH)]
            Qcur = q_proj_main(0, 0)
            Qts = [q_proj_tail(0, 0), None]  # tail tiles by pair parity
            Qnext = [None]
            pending_out = [None]
            UAPs = [None, None]  # pair-grouped U tiles by pair parity

            for hi, (nch, h) in enumerate(heads):
                Qm = Qcur
                Qt = Qts[(h // 2) % 2]
                tb = 64 * (h % 2)
                Km, Kt = Kms[h], Kts[h // 2]
                nxt = heads[hi + 1] if hi + 1 < len(heads) else None

                pA = ppu.tile([P, 512], F32, name=f"pa{nch}{h}", tag="pu")
                pB = ppu.tile([P, 512], F32, name=f"pb{nch}{h}", tag="pu")

                last_head = False  # A/B: unsplit tail
                is_final = hi == len(heads) - 1

                def u_mms(m, Em, g=None, pA=pA, pB=pB, h=h):
                    gg = ds(0, 512) if g is None else g
                    nc.tensor.matmul(
                        pA[:, gg],
                        lhsT=Vseq[m][:, SEG * h : SEG * h + 128],
                        rhs=Em[:, gg],
                        start=(m == 0), stop=(m == 7),
                        skip_group_check=g is not None,
                    )
                    # [d2 48 | pad | ones@64 | pad]: row 64 = softmax row sums
                    nc.tensor.matmul(
                        pB[0:96, gg],
                        lhsT=Vseq[m][:, SEG * h + 128 : SEG * h + 224],
                        rhs=Em[:, gg],
                        start=(m == 0), stop=(m == 7),
                        skip_group_check=g is not None,
                    )

                Eq = []
                for m in range(8):
                    pS = pps.tile([P, 512], F32, name=f"s{nch}{h}{m}", tag="ps")
                    nc.tensor.matmul(
                        pS[:], lhsT=Km[:, ts(m, P)], rhs=Qm[:],
                        start=True, stop=False,
                    )
                    nc.tensor.matmul(
                        pS[:], lhsT=Kt[tb : tb + 48, ts(m, P)],
                        rhs=Qt[tb : tb + 48, :],
                        start=False, stop=True,
                    )
                    E = epool.tile([P, 512], BF16, name=f"e{nch}{h}{m}", tag="e")
                    nc.scalar.activation(
                        E[:], pS[:], mybir.ActivationFunctionType.Exp,
                        scale=SCALE,
                    )
                    Eq.append(E)
                    if m in (0, 2, 6):
                        run_side_job()
                    if m == 3 and nxt is not None:
                        Qnext[0] = q_proj_main(*nxt)
                    if m == 5 and pending_out[0] is not None:
                        pending_out[0]()
                        pending_out[0] = None
                    if m == 5 and nxt is not None and nxt[1] % 2 == 0:
                        Qts[(nxt[1] // 2) % 2] = q_proj_tail(*nxt)
                    if m >= LAG:
                        u_mms(m - LAG, Eq[m - LAG],
                              ds(0, 256) if last_head else None)

                # normalize: reciprocal of row sums (pB row 64), broadcast
                # across partitions, then scale each 32-aligned d block into
                # the pair-grouped UAP tiles: rows 0:48 = even head, 64:112 =
                # odd head.  All sources/destinations sit at 32-aligned
                # partition bases.
                pr, sb = h // 2, 64 * (h % 2)
                if h % 2 == 0:
                    UAPc = [
                        uapp.tile([P, 512], BF16, name=f"up{nch}{pr}{d}",
                                  tag=f"up{d}{pr % 2}")
                        for d in range(3)
                    ]
                    UAPs[pr % 2] = UAPc
                UAP = UAPs[pr % 2]
                halves = [ds(0, 512)] if not last_head else [
                    ds(0, 256), ds(256, 256)]
                rr = rrp.tile([P, 512], F32, name=f"rr{nch}{h}", tag="rr")
                Rsb = rrp.tile([P, 512], F32, name=f"rs{nch}{h}", tag="rs")

                def normalize(g, pA=pA, pB=pB, UAP=UAP, sb=sb):
                    nc.vector.reciprocal(out=rr[0:1, g], in_=pB[64:65, g])
                    nc.gpsimd.partition_broadcast(Rsb[:, g], rr[0:1, g])
                    # 64-row blocks: rows 48:64 of each source are zero (the
                    # V-segment pad columns), giving the UAP pad rows true
                    # zeros at no extra cost.
                    for d, (src, lo) in enumerate(
                        [(pA, 0), (pA, 64), (pB, 0)]
                    ):
                        nc.vector.tensor_mul(
                            out=UAP[d][sb : sb + 64, g],
                            in0=src[lo : lo + 64, g], in1=Rsb[0:64, g],
                        )

                # pair-grouped output projection: after pairs (0,1) and
                # (2,3) complete, accumulate Wo over both pairs in PSUM and
                # drain into the SBUF output accumulator.
                def out_stage(nch, stage, UAP2, gs=(ds(0, 512),)):
                    for g in gs:
                        pO = [
                            ppu.tile([P, 512], F32, name=f"po{nch}{stage}{d}",
                                     tag="pu")
                            for d in range(3)
                        ]
                        for d in range(3):
                            for j in range(2):
                                nc.tensor.matmul(
                                    pO[d][:, g], lhsT=WFT[:, 2 * stage + j, :],
                                    rhs=UAP2[j][d][:, g],
                                    start=(j == 0), stop=(j == 1),
                                )
                        eng_dma = [nc.sync, nc.scalar, nc.gpsimd]
                        for d in range(3):
                            osl = OutSB[:, d, ds(512 * nch, 512)][:, g]
                            if stage == 0:
                                nc.vector.tensor_copy(out=osl, in_=pO[d][:, g])
                            else:
                                nc.vector.tensor_add(
                                    out=osl, in0=osl, in1=pO[d][:, g])
                                eng_dma[d].dma_start(
                                    out.ap()[:, d, ds(512 * nch, 512)][:, g],
                                    osl,
                                )

                if not last_head:
                    for m in range(8 - LAG, 8):
                        u_mms(m, Eq[m])
                    Qcur = Qnext[0]
                    normalize(halves[0])
                else:
                    # finish the first query half early, overlap its
                    # normalize with the second half's U accumulation
                    for m in range(8 - LAG, 8):
                        u_mms(m, Eq[m], halves[0])
                    normalize(halves[0])
                    for m in range(8):
                        u_mms(m, Eq[m], halves[1])
                    normalize(halves[1])
                if h % 4 == 3:  # pairs (0,1) done at h=3, (2,3) at h=7
                    UAP2 = [UAPs[(pr - 1) % 2], UAPs[pr % 2]]
                    if not last_head:
                        def out_proj(nch=nch, stage=pr // 2, UAP2=UAP2):
                            out_stage(nch, stage, UAP2)
                        pending_out[0] = out_proj
                    else:
                        out_stage(nch, pr // 2, UAP2, gs=halves)

            if pending_out[0] is not None:
                pending_out[0]()

    nc.compile()
    return nc


def _prep_weights(Wq, Wk, Wv, Wo):
    def compact_qk(W):
        # [8 head main blocks (48 each) | 4 pair tail blocks (48|48)] — both
        # sections happen to be W^T in natural column order; the kernel
        # builds the zero-padded stacked form on-chip.
        Wt = np.ascontiguousarray(W.T).astype(np.float32)  # (128 c, 384 o)
        return np.concatenate([Wt, Wt], axis=1)  # (P, WCMP)

    WoT = np.ascontiguousarray(Wo.T).astype(np.float32)  # (384 o, 128 co)
    # pair-stacked: rows 0:48 = even head's block, 64:112 = odd head's
    wf = np.zeros((P, NH // 2, P), np.float32)
    for p in range(NH // 2):
        wf[0:48, p] = WoT[FH * 2 * p : FH * 2 * p + FH]
        wf[64:112, p] = WoT[FH * (2 * p + 1) : FH * (2 * p + 1) + FH]
    return (
        compact_qk(Wq),
        compact_qk(Wk),
        np.ascontiguousarray(Wv.T).astype(np.float32),
        np.ascontiguousarray(wf.reshape(P, (NH // 2) * P)),
    )


_CACHED_NC = None


def _make_in_maps(vn_x, Wq, Wk, Wv, Wo):
    import ml_dtypes

    wqs, wks, wvt, wf = _prep_weights(
        np.asarray(Wq), np.asarray(Wk), np.asarray(Wv), np.asarray(Wo)
    )
    wf = np.ascontiguousarray(wf.astype(ml_dtypes.bfloat16))
    vn_x = np.asarray(vn_x)
    maps = []
    for b in range(NCORES):
        xb = vn_x[b]
        # SBUF/DRAM layout: [WVT | X0 | X1 | WKc | X2 | WQc]
        packed = np.concatenate(
            [wvt, xb[:, 0], xb[:, 1], wks, xb[:, 2], wqs], axis=1
        ).astype(ml_dtypes.bfloat16)
        assert packed.shape == (P, PACK1W), packed.shape
        maps.append(
            {"packed": np.ascontiguousarray(packed), "wfin": wf}
        )
    return maps


def kernel(vn_x, Wq, Wk, Wv, Wo):
    global _CACHED_NC
    if _CACHED_NC is None:
        _CACHED_NC = _build_program()
    nc = _CACHED_NC

    in_maps = _make_in_maps(vn_x, Wq, Wk, Wv, Wo)
    res = run_bass_kernel_spmd(nc, in_maps, core_ids=list(range(NCORES)))
    out = np.stack([res.results[b]["out"] for b in range(NCORES)])
    return out
